# revision 8
# baseline (speedup 1.0000x reference)
"""ContextEncoder Trainium2 kernel: masked MHA + mean-pool + layernorm.

Full inputs -> (theta [4,1024,1024] f32, context [4,1024] f32).

Sharding: 8 cores = batch b (4) x head-half hh (2). Each core computes
theta_b (identity * mask) and the pooled context for its 8 heads
(512 of 1024 context dims). Host does the final [4,1024] layernorm+clip.

Device formulation (per core, all matmuls float32r, 1 cyc/row):
  - QK^T projection in transposed layout: qkT[j', t] = sum_d w[d, j'] xT[d, t]
  - V in natural layout:  v[t, j] = sum_d xT[d, t-block] wv[d, j]
  - scoresT[tk, tq] = K_h^T-stationary @ Q_h^T-moving (contraction j=64)
  - softmax without max-subtraction (scores bounded):
      es = exp(scoresT * 0.125 + keymask_col)        (ACT, fused mask)
      at = es * wexp_window                          (DVE; exp(rel-pos bias)
                                                      as a host-built sliding
                                                      Toeplitz window tile)
  - ctxT[j, tq] (+ denom row via ones-column) = [V_h|1]-stationary @ at-moving
  - g[tq] = u[tq] / denom[tq]  (u = (t<L)/L), broadcast via DRAM roundtrip
  - pooled[j] = sum_tq ctxT[j, tq] * g[tq]           (DVE mul + reduce)
"""

import json

import numpy as np

B, T, D = 4, 1024, 1024
H = 16
HD = D // H  # 64
HPC = 8      # heads per core
MAX_SEQ_LEN = 1024
LN_EPS = 1e-5
NEG = -30000.0

_CACHE = {}


def _legalize_nc(nc):
    """This walrus build allows one sync wait per instruction; move extra
    waits onto wait-only Drain carriers inserted before the instruction."""
    j = nc.to_json()
    for fn in j["functions"]:
        for blk in fn["blocks"]:
            out = []
            for inst in blk["instructions"]:
                si = inst.get("sync_info")
                waits = (si or {}).get("on_wait") or []
                if len(waits) > 1:
                    for k, w in enumerate(waits[:-1]):
                        out.append({
                            "debug": inst.get("debug", 0),
                            "engine": inst["engine"],
                            "ins": [],
                            "is_reset_sema": False,
                            "name": f"{inst['name']}-w{k}",
                            "opcode": "Drain",
                            "outs": [],
                            "sync_info": {"on_update": [], "on_wait": [w]},
                        })
                    si["on_wait"] = [waits[-1]]
                out.append(inst)
            blk["instructions"] = out
    data = json.dumps(j).encode()
    nc.to_json_bytes = lambda: data
    return nc


def _build_program():
    import concourse.bass as bass
    import concourse.mybir as mybir
    from concourse.tile import TileContext

    f32 = mybir.dt.float32
    f32r = mybir.dt.float32r
    Exp = mybir.ActivationFunctionType.Exp
    X = mybir.AxisListType.X

    nc = bass.Bass()
    x_ext = nc.declare_dram_parameter("x", [T, D], f32, isOutput=False)
    xT_ext = nc.declare_dram_parameter("xT", [D, T], f32, isOutput=False)
    w_ext = nc.declare_dram_parameter("w", [D, 1536], f32, isOutput=False)
    bqk_ext = nc.declare_dram_parameter("bqk", [1024], f32, isOutput=False)
    bv_ext = nc.declare_dram_parameter("bv", [512], f32, isOutput=False)
    wexp_ext = nc.declare_dram_parameter("wexp", [HPC, 128, 2047], f32, isOutput=False)
    km_ext = nc.declare_dram_parameter("km", [T], f32, isOutput=False)
    mcol_ext = nc.declare_dram_parameter("mcol", [T], f32, isOutput=False)
    urow_ext = nc.declare_dram_parameter("urow", [T], f32, isOutput=False)
    theta_ext = nc.declare_dram_parameter("theta", [T, D], f32, isOutput=True)
    ctxp_ext = nc.declare_dram_parameter("ctxp", [512], f32, isOutput=True)

    with TileContext(nc) as tc:
        with tc.tile_pool(name="const", bufs=1) as cp, \
             tc.tile_pool(name="xin", bufs=3) as xin_p:

            qkT_sb = cp.tile([128, 8, 1024], f32r)
            v_sb = cp.tile([128, 8, HPC, 128], f32r)
            bqk_sb = cp.tile([128, 8], f32)
            bv_sb = cp.tile([128, 512], f32)
            km_sb = cp.tile([128, 8], f32)
            mcol_sb = cp.tile([128, 8], f32)
            urow_sb = cp.tile([64, 1024], f32)

            # column-major [p, chunk] views of 1024-vectors
            for vec, tile in ((bqk_ext, bqk_sb), (km_ext, km_sb), (mcol_ext, mcol_sb)):
                nc.sync.dma_start(
                    out=tile, in_=bass.AP(tensor=vec, offset=0,
                                          ap=[[1, 128], [128, 8]]))
            nc.sync.dma_start(out=bv_sb, in_=bv_ext[:].partition_broadcast(128))
            nc.sync.dma_start(out=urow_sb, in_=urow_ext[:].partition_broadcast(64))
            ones_sb = cp.tile([128, 64], f32)
            nc.vector.memset(ones_sb[:], 1.0)
            for tb in range(8):
                for hh in range(HPC):
                    nc.vector.tensor_copy(v_sb[:, tb, hh, 64:128], ones_sb[:])

            # ---- theta = x * mask (independent stream, in-place) ----
            for i in range(8):
                xt = xin_p.tile([128, 1024], f32)
                nc.sync.dma_start(
                    out=xt, in_=bass.AP(tensor=x_ext, offset=i * 128 * 1024,
                                        ap=[[1024, 128], [1, 1024]]))
                nc.vector.tensor_scalar_mul(xt[:], xt[:], mcol_sb[:, i:i + 1])
                nc.sync.dma_start(
                    out=bass.AP(tensor=theta_ext, offset=i * 128 * 1024,
                                ap=[[1024, 128], [1, 1024]]),
                    in_=xt)

            # ---- phase 1: projections (w/xT freed afterwards) ----
            with tc.tile_pool(name="proj", bufs=1) as pp, \
                 tc.tile_pool(name="qkps", bufs=3, space="PSUM") as qk_ps, \
                 tc.tile_pool(name="vps", bufs=2, space="PSUM") as v_ps:
                w_sb = pp.tile([128, 8, 1536], f32r)
                xT_sb = pp.tile([128, 8, 1024], f32r)
                for dc in range(8):
                    nc.sync.dma_start(
                        out=w_sb[:, dc, :],
                        in_=bass.AP(tensor=w_ext, offset=dc * 128 * 1536,
                                    ap=[[1536, 128], [1, 1536]]).bitcast(f32r))
                    nc.sync.dma_start(
                        out=xT_sb[:, dc, :],
                        in_=bass.AP(tensor=xT_ext, offset=dc * 128 * 1024,
                                    ap=[[1024, 128], [1, 1024]]).bitcast(f32r))
                for jb in range(8):
                    for th in range(2):
                        ps = qk_ps.tile([128, 512], f32)
                        for dc in range(8):
                            nc.tensor.matmul(
                                ps[:],
                                w_sb[:, dc, jb * 128:(jb + 1) * 128],
                                xT_sb[:, dc, th * 512:(th + 1) * 512],
                                start=(dc == 0), stop=(dc == 7))
                        nc.vector.tensor_scalar_add(
                            qkT_sb[:, jb, th * 512:(th + 1) * 512],
                            ps[:], bqk_sb[:, jb:jb + 1])
                for tb in range(8):
                    ps = v_ps.tile([128, 512], f32)
                    for dc in range(8):
                        nc.tensor.matmul(
                            ps[:],
                            xT_sb[:, dc, tb * 128:(tb + 1) * 128],
                            w_sb[:, dc, 1024:1536],
                            start=(dc == 0), stop=(dc == 7))
                    nc.vector.tensor_add(
                        v_sb[:, tb, :, 0:64],
                        ps[:].rearrange("p (h j) -> p h j", h=HPC),
                        bv_sb[:].rearrange("p (h j) -> p h j", h=HPC))

            # ---- phase 2: attention per head ----
            with tc.tile_pool(name="wexp", bufs=2) as we_p, \
                 tc.tile_pool(name="es", bufs=3) as es_p, \
                 tc.tile_pool(name="at", bufs=3) as at_p, \
                 tc.tile_pool(name="fin", bufs=2) as fin_p, \
                 tc.tile_pool(name="sps", bufs=3, space="PSUM") as s_ps, \
                 tc.tile_pool(name="cps", bufs=4, space="PSUM") as c_ps:
                for h in range(HPC):
                    we = we_p.tile([128, 2047], f32)
                    nc.sync.dma_start(out=we, in_=wexp_ext[h])
                    poff = (h % 2) * 64
                    jbq = h // 2
                    jbk = 4 + h // 2
                    ctx = [c_ps.tile([128, 512], f32, name=f"ctx{h}_{i}", tag="ctx") for i in range(2)]
                    for tc_i in range(8):
                        for th in range(2):
                            sps = s_ps.tile([128, 512], f32)
                            nc.tensor.matmul(
                                sps[:],
                                qkT_sb[poff:poff + 64, jbk,
                                       tc_i * 128:(tc_i + 1) * 128],
                                qkT_sb[poff:poff + 64, jbq,
                                       th * 512:(th + 1) * 512],
                                start=True, stop=True)
                            es = es_p.tile([128, 512], f32)
                            nc.scalar.activation(
                                es[:], sps[:], Exp,
                                bias=km_sb[:, tc_i:tc_i + 1], scale=0.125)
                            at = at_p.tile([128, 512], f32r)
                            c0 = 1023 - 128 * tc_i + th * 512
                            nc.vector.tensor_mul(at[:], es[:], we[:, c0:c0 + 512])
                            nc.tensor.matmul(
                                ctx[th][:], v_sb[:, tc_i, h, 0:128], at[:],
                                start=(tc_i == 0), stop=(tc_i == 7))
                    # pooled_h[j] = sum_tq ctxT[j,tq] * u[tq]/denom[tq];
                    # psum rows 64:128 hold denom replicated (ones columns)
                    ctxn = fin_p.tile([64, 1024], f32)
                    for th in range(2):
                        rec = fin_p.tile([64, 512], f32)
                        nc.vector.reciprocal(rec[:], ctx[th][64:128, :])
                        recu = fin_p.tile([64, 512], f32)
                        nc.vector.tensor_mul(
                            recu[:], rec[:], urow_sb[:, th * 512:(th + 1) * 512])
                        nc.vector.tensor_mul(
                            ctxn[:, th * 512:(th + 1) * 512],
                            ctx[th][0:64, :], recu[:])
                    pooled = fin_p.tile([64, 1], f32)
                    nc.vector.reduce_sum(pooled[:], ctxn[:], axis=X)
                    nc.sync.dma_start(
                        out=bass.AP(tensor=ctxp_ext, offset=h * 64, ap=[[1, 64]]),
                        in_=pooled[:, 0])

    _legalize_nc(nc)
    return nc


def _prep_inputs(x, lengths, w_qkv, b_qkv, rel_pos):
    x = np.asarray(x, dtype=np.float32)
    lengths = np.asarray(lengths, dtype=np.int32)
    w_qkv = np.asarray(w_qkv, dtype=np.float32)
    b_qkv = np.asarray(b_qkv, dtype=np.float32)
    rel_pos = np.asarray(rel_pos, dtype=np.float32)

    # per-half packed weights/biases and exp(rel-pos) window tiles
    halves = []
    p_idx = np.arange(128)[:, None]
    c_idx = np.arange(2047)[None, :]
    gidx = c_idx - p_idx + 127  # into rpad, length 127 + 2047
    for hh in range(2):
        cols = slice(hh * 512, (hh + 1) * 512)
        w_half = np.ascontiguousarray(np.concatenate(
            [w_qkv[:, 0:1024][:, cols],
             w_qkv[:, 1024:2048][:, cols],
             w_qkv[:, 2048:3072][:, cols]], axis=1))
        bqk = np.ascontiguousarray(np.concatenate(
            [b_qkv[0:1024][cols], b_qkv[1024:2048][cols]]))
        bv = np.ascontiguousarray(b_qkv[2048:3072][cols])
        heads = range(hh * HPC, (hh + 1) * HPC)
        wexp = np.empty((HPC, 128, 2047), np.float32)
        for i, hg in enumerate(heads):
            rpad = np.ones(127 + 2047, np.float32)
            rpad[127:] = np.exp(rel_pos[:, hg])
            wexp[i] = rpad[gidx]
        halves.append((w_half, bqk, bv, wexp))

    t_ar = np.arange(T)
    in_maps = []
    for c in range(8):
        b, hh = c // 2, c % 2
        L = int(lengths[b])
        maskf = (t_ar < L).astype(np.float32)
        km = np.where(t_ar < L, 0.0, NEG).astype(np.float32)
        urow = (maskf / max(L, 1)).astype(np.float32)
        w_half, bqk, bv, wexp = halves[hh]
        in_maps.append({
            "x": x[b],
            "xT": np.ascontiguousarray(x[b].T),
            "w": w_half,
            "bqk": bqk,
            "bv": bv,
            "wexp": wexp,
            "km": km,
            "mcol": maskf,
            "urow": urow,
        })
    return in_maps


def kernel(x, lengths, w_qkv, b_qkv, rel_pos):
    if "nc" not in _CACHE:
        _CACHE["nc"] = _build_program()
    nc = _CACHE["nc"]

    in_maps = _prep_inputs(x, lengths, w_qkv, b_qkv, rel_pos)
    from concourse.bass_utils import run_bass_kernel_spmd
    res = run_bass_kernel_spmd(nc, in_maps, list(range(8))).results

    theta = np.stack([res[2 * b]["theta"] for b in range(B)])
    ctx = np.stack([
        np.concatenate([res[2 * b]["ctxp"], res[2 * b + 1]["ctxp"]])
        for b in range(B)
    ]).astype(np.float64)
    mu = ctx.mean(axis=-1, keepdims=True)
    var = ((ctx - mu) ** 2).mean(axis=-1, keepdims=True)
    ctx = (ctx - mu) / np.sqrt(var + LN_EPS)
    ctx = np.clip(ctx, -10.0, 10.0).astype(np.float32)
    return theta, ctx
